# revision 10
# baseline (speedup 1.0000x reference)
"""MAD predictor (retrieval_knn) — Trainium2 Bass/Tile kernel on 8 NeuronCores.

v3 (node-dedup, two stages):
Host packs edges into 8 bins of 512 via connected-component clustering so
each core's 512 edges touch only ~700 distinct nodes (6 row-tiles vs 8).
Stage 1 (per head, per node-tile): S = 2x·e - |e|^2 via one fp8 DoubleRow
matmul per chunk (K=256: 128 embedding dims + the -|e|^2 bias split into
3 fp8 components on spare contraction slots), 4-deep 1024-wide PSUM
pipeline, paired-block MAX8 from the f32 SBUF copy, merge -> top-9,
FIND_INDEX8 ->
neighbor ids; neighbor embedding rows gathered; per-node d2 -> w=exp(1-d)
batched per head. Results land in per-head DRAM tables (w / ids / e-rows)
via a DRAM tile pool (RAW tracked by the tile framework).
Stage 2 (per head, per edge-tile): pure-gpsimd pipeline gathers each
edge's endpoint-node rows from the tables by host-known slot offsets,
gathers adjacency bits, computes EG = e_s·g via bcast-mult + add-tree.
Tail: logits/softmin/sigmoid once per edge-tile.
"""

import sys
from contextlib import ExitStack

for _p in ('/opt/trn_rl_repo', '/root/.axon_site/_ro/trn_rl_repo'):
    if _p not in sys.path:
        sys.path.append(_p)

import numpy as np
import ml_dtypes

import concourse.bass as bass
import concourse.bacc as bacc
import concourse.mybir as mybir
from concourse.tile import TileContext
from concourse.bass_utils import run_bass_kernel_spmd

BF16 = mybir.dt.bfloat16
F32 = mybir.dt.float32
FP8 = mybir.dt.float8e4
U32 = mybir.dt.uint32
U8 = mybir.dt.uint8
P = 128
NEG_BIG = -3.0e38
bf = ml_dtypes.bfloat16
f8 = ml_dtypes.float8_e4m3

H, N, D = 4, 10000, 128
B, NCORES = 4096, 8
NB = B // NCORES          # 512 edges per core
RT = NB // P              # 4 edge-tiles of 128
NSENT = 8
GRP = 1024
MMC = 512
KD = 8 * D                # 1024 elems of neighbor-rows per node
KR = KD + 16              # etab row: erows + idx8 (8 u32 as 16 bf16)


def _chunks(total, step):
    out, o = [], 0
    while o < total:
        out.append((o, min(step, total - o)))
        o += step
    return out


def build_kernel(u, NT):
    SLT = NT * P
    grps = _chunks(N, GRP)
    NG = len(grps)

    nc = bacc.Bacc("TRN2", target_bir_lowering=False, debug=False,
                   enable_asserts=True, num_devices=NCORES)

    eT8 = nc.declare_dram_parameter("eT8", [H, D, 2 * N], FP8, isOutput=False)
    xT8 = nc.declare_dram_parameter("xT8", [H, D, 2 * SLT], FP8,
                                    isOutput=False)
    xgp = nc.declare_dram_parameter("xgp", [RT, P, 8], F32, isOutput=False)
    prow = nc.declare_dram_parameter("prow", [P, 1], U32, isOutput=False)
    tofs = nc.declare_dram_parameter("tofs", [P, 2 * RT], U32, isOutput=False)
    eofs = nc.declare_dram_parameter("eofs", [P, 2 * RT], U32, isOutput=False)
    grows = nc.declare_dram_parameter("grows", [H, 2, NB, D], BF16,
                                      isOutput=False)
    emb = {h: nc.declare_dram_parameter(f"emb_{h}", [N, D], BF16,
                                        isOutput=False) for h in range(H)}
    astage = {}
    for bu in range(2):
        for rt in range(RT):
            astage[(bu, rt)] = nc.declare_dram_parameter(
                f"astage_{bu}_{rt}", [P, N], U8, isOutput=False)
    out_p = nc.declare_dram_parameter("out", [NB, 1], F32, isOutput=True)

    with TileContext(nc) as tc, ExitStack() as ctx:
        pconst = ctx.enter_context(tc.tile_pool(name="const", bufs=1))
        pbig = ctx.enter_context(tc.tile_pool(name="big", bufs=2))
        psmall = ctx.enter_context(tc.tile_pool(name="small", bufs=3))
        pstage = ctx.enter_context(tc.tile_pool(name="stage", bufs=1))
        ppsum = ctx.enter_context(tc.tile_pool(name="psum", bufs=2,
                                               space="PSUM"))
        pdram = ctx.enter_context(tc.tile_pool(name="dram", bufs=1,
                                               space="DRAM"))

        # per-head stage-1 -> stage-2 tables (flat, element offsets)
        wtab = {h: pdram.tile([P, NT * 8], F32, tag=f"wtab{h}",
                              name=f"wtab{h}") for h in range(H)}
        etab = {h: pdram.tile([P, NT * KR], BF16, tag=f"etab{h}",
                              name=f"etab{h}") for h in range(H)}

        selfpad = pconst.tile([P, 8], F32)
        nc.vector.memset(selfpad[:], NEG_BIG)
        prow_t = pconst.tile([P, 1], U32)
        nc.sync.dma_start(out=prow_t[:], in_=prow[:, :])
        tofs_t = pconst.tile([P, 2 * RT], U32)
        nc.sync.dma_start(out=tofs_t[:], in_=tofs[:, :])
        eofs_t = pconst.tile([P, 2 * RT], U32)
        nc.sync.dma_start(out=eofs_t[:], in_=eofs[:, :])

        ws = {rt: pstage.tile([P, 64], F32, tag=f"ws{rt}", name=f"ws{rt}")
              for rt in range(RT)}
        egs = {rt: pstage.tile([P, 64], F32, tag=f"egs{rt}", name=f"egs{rt}")
               for rt in range(RT)}
        a8s = {rt: pstage.tile([P, 64], U8, tag=f"a8s{rt}", name=f"a8s{rt}")
               for rt in range(RT)}
        xgs = {rt: pstage.tile([P, 8], F32, tag=f"xgs{rt}", name=f"xgs{rt}")
               for rt in range(RT)}
        for rt in range(RT):
            nc.sync.dma_start(out=xgs[rt][:], in_=xgp[rt])

        # PE warmup
        wsrc = pconst.tile([P, 512], BF16)
        nc.vector.memset(wsrc[:], 0.001)
        wps = ppsum.tile([P, GRP], F32, tag="psS", bufs=4)
        for _ in range(24):
            nc.tensor.matmul(wps[:, :512], lhsT=wsrc[:, :P], rhs=wsrc[:],
                             start=True, stop=True)
        wout = pconst.tile([1, 1], F32)
        nc.vector.tensor_copy(wout[:], wps[:1, :1])

        def emit_tail(rt):
            t1 = psmall.tile([P, 64], F32, tag="t1")
            nc.vector.scalar_tensor_tensor(
                out=t1[:], in0=a8s[rt][:], scalar=2.0 * u, in1=egs[rt][:],
                op0=mybir.AluOpType.mult, op1=mybir.AluOpType.subtract)
            xgu = psmall.tile([P, 8], F32, tag="xgu")
            nc.vector.tensor_scalar_add(xgu[:], xgs[rt][:], -u)
            l3 = psmall.tile([P, 64], F32, tag="l3")
            nc.vector.tensor_tensor(
                out=l3[:].rearrange("p (g k) -> p g k", g=8),
                in0=t1[:].rearrange("p (g k) -> p g k", g=8),
                in1=xgu[:].rearrange("p (g o) -> p g o",
                                     o=1).to_broadcast([P, 8, 8]),
                op=mybir.AluOpType.add)
            wl = psmall.tile([P, 64], F32, tag="wl")
            nc.vector.tensor_tensor(out=wl[:], in0=ws[rt][:], in1=l3[:],
                                    op=mybir.AluOpType.mult)
            swk = psmall.tile([P, 8], F32, tag="swk")
            nc.vector.tensor_reduce(
                swk[:], ws[rt][:].rearrange("p (g k) -> p g k", g=8),
                axis=mybir.AxisListType.X, op=mybir.AluOpType.add)
            swlk = psmall.tile([P, 8], F32, tag="swlk")
            nc.vector.tensor_reduce(
                swlk[:], wl[:].rearrange("p (g k) -> p g k", g=8),
                axis=mybir.AxisListType.X, op=mybir.AluOpType.add)
            swh = psmall.tile([P, 4], F32, tag="swh")
            nc.vector.tensor_reduce(
                swh[:], swk[:].rearrange("p (h b) -> p h b", h=4),
                axis=mybir.AxisListType.X, op=mybir.AluOpType.add)
            swlh = psmall.tile([P, 4], F32, tag="swlh")
            nc.vector.tensor_reduce(
                swlh[:], swlk[:].rearrange("p (h b) -> p h b", h=4),
                axis=mybir.AxisListType.X, op=mybir.AluOpType.add)
            den = psmall.tile([P, 4], F32, tag="den")
            nc.vector.tensor_scalar_add(den[:], swh[:], float(NSENT))
            rec = psmall.tile([P, 4], F32, tag="rec")
            nc.vector.reciprocal(rec[:], den[:])
            smin = psmall.tile([P, 4], F32, tag="smin")
            nc.vector.tensor_tensor(out=smin[:], in0=swlh[:], in1=rec[:],
                                    op=mybir.AluOpType.mult)
            ssum = psmall.tile([P, 1], F32, tag="ssum")
            nc.vector.tensor_reduce(
                ssum[:], smin[:].rearrange("p (o f) -> p o f", o=1),
                axis=mybir.AxisListType.X, op=mybir.AluOpType.add)
            sig = psmall.tile([P, 1], F32, tag="sig")
            nc.scalar.activation(sig[:], ssum[:],
                                 mybir.ActivationFunctionType.Sigmoid,
                                 scale=1.0 / H)
            nc.sync.dma_start(out=out_p[rt * P:(rt + 1) * P, :], in_=sig[:])

        for h in range(H):
            eTh = pbig.tile([D, 2 * N], FP8, tag="eTh")
            nc.sync.dma_start(out=eTh[:], in_=eT8[h])
            xTh = psmall.tile([D, 2 * SLT], FP8, tag="xTh", bufs=2)
            nc.sync.dma_start(out=xTh[:], in_=xT8[h])
            d2h = psmall.tile([P, NT * 8], F32, tag="d2h", bufs=2)

            etabv = etab[h][:, :]
            wtabv = wtab[h][:, :]

            # ---- stage 1: per node-tile top-9 + neighbor rows ----
            for nt in range(NT):
                S_sb = pbig.tile([P, N], F32, tag="S_sb", bufs=2)
                cand = psmall.tile([P, 8 * ((NG + 1) // 2)], F32,
                                   tag="cand")
                for gi, (go, gw) in enumerate(grps):
                    psS = ppsum.tile([P, GRP], F32, tag="psS", bufs=4)
                    xv = xTh[:].rearrange("d (two s) -> d two s", two=2)
                    ev = eTh[:].rearrange("d (two n) -> d two n", two=2)
                    for (co, cw) in _chunks(gw, MMC):
                        nc.tensor.matmul(
                            psS[:, co:co + cw],
                            lhsT=xv[:, :, nt * P:(nt + 1) * P],
                            rhs=ev[:, :, go + co:go + co + cw],
                            start=True, stop=True,
                            perf_mode=mybir.MatmulPerfMode.DoubleRow)
                    nc.scalar.copy(S_sb[:, go:go + gw], psS[:, :gw])
                    if gi % 2 == 1 or gi == NG - 1:
                        bo = (gi // 2) * 2 * GRP
                        bw = go + gw - bo
                        nc.vector.max(out=cand[:, (gi // 2) * 8:
                                                 (gi // 2) * 8 + 8],
                                      in_=S_sb[:, bo:bo + bw])

                m1 = psmall.tile([P, 8], F32, tag="m1")
                nc.vector.max(out=m1[:], in_=cand[:])
                nc.vector.tensor_copy(selfpad[:, 0:1], m1[:, 0:1])
                candz = psmall.tile([P, 8 * ((NG + 1) // 2)], F32,
                                    tag="candz")
                nc.vector.match_replace(out=candz[:], in_to_replace=selfpad[:],
                                        in_values=cand[:], imm_value=NEG_BIG)
                nv = psmall.tile([P, 8], F32, tag="nv")
                nc.vector.max(out=nv[:], in_=candz[:])
                nc.vector.tensor_tensor(
                    out=d2h[:, nt * 8:(nt + 1) * 8],
                    in0=m1[:, 0:1].to_broadcast([P, 8]), in1=nv[:],
                    op=mybir.AluOpType.subtract)
                idx = psmall.tile([P, 8], U32, tag="idx", bufs=8)
                nc.vector.max_index(idx[:], nv[:], S_sb[:])
                nc.sync.dma_start(
                    out=etabv[:, nt * KR + KD:nt * KR + KR].bitcast(U32),
                    in_=idx[:])

                erows = psmall.tile([P, KD], BF16, tag="erows", bufs=6)
                for kk in range(8):
                    nc.gpsimd.indirect_dma_start(
                        out=erows[:, kk * D:(kk + 1) * D], out_offset=None,
                        in_=emb[h][:, :],
                        in_offset=bass.IndirectOffsetOnAxis(
                            ap=idx[:, kk:kk + 1], axis=0))
                nc.sync.dma_start(out=etabv[:, nt * KR:nt * KR + KD],
                                  in_=erows[:])

            dsth = psmall.tile([P, NT * 8], F32, tag="dsth")
            nc.scalar.sqrt(dsth[:], d2h[:])
            wh = psmall.tile([P, NT * 8], F32, tag="wh")
            nc.scalar.activation(wh[:], dsth[:],
                                 mybir.ActivationFunctionType.Exp,
                                 bias=1.0, scale=-1.0)
            nc.sync.dma_start(out=wtabv[:, :], in_=wh[:])

            # ---- stage 2: per edge-tile gathers + EG ----
            if h == H - 1:
                s2pairs = [(bu, rt) for rt in range(RT) for bu in range(2)]
            else:
                s2pairs = [(bu, rt) for bu in range(2) for rt in range(RT)]
            for bu, rt in s2pairs:
                    hb = h * 2 + bu
                    it = bu * RT + rt
                    nc.gpsimd.indirect_dma_start(
                        out=ws[rt][:, hb * 8:(hb + 1) * 8], out_offset=None,
                        in_=wtab[h][:, :],
                        in_offset=bass.IndirectOffsetOnAxis(
                            ap=tofs_t[:, it:it + 1], axis=1))
                    erows2 = psmall.tile([P, KR], BF16, tag="erows2")
                    nc.gpsimd.indirect_dma_start(
                        out=erows2[:], out_offset=None, in_=etab[h][:, :],
                        in_offset=bass.IndirectOffsetOnAxis(
                            ap=eofs_t[:, it:it + 1], axis=1))
                    idx8 = erows2[:, KD:KR].bitcast(U32)

                    gtile = psmall.tile([P, D], BF16, tag="gtile")
                    nc.sync.dma_start(
                        out=gtile[:],
                        in_=grows[h, bu, rt * P:(rt + 1) * P, :])
                    prod = psmall.tile([P, KD], F32, tag="prod")
                    e3 = erows2[:, 0:KD].rearrange("p (k d) -> p k d", k=8)
                    g3 = gtile[:].rearrange("p (o d) -> p o d",
                                            o=1).to_broadcast([P, 8, D])
                    p3 = prod[:].rearrange("p (k d) -> p k d", k=8)
                    if h == H - 1:
                        # DVE is idle in the endgame: mult (bf16, 2x) + reduce
                        prodb = psmall.tile([P, KD], BF16, tag="prodb")
                        pb3 = prodb[:].rearrange("p (k d) -> p k d", k=8)
                        nc.vector.tensor_tensor(out=pb3, in0=e3, in1=g3,
                                                op=mybir.AluOpType.mult)
                        nc.vector.tensor_reduce(
                            egs[rt][:, hb * 8:(hb + 1) * 8], pb3,
                            axis=mybir.AxisListType.X, op=mybir.AluOpType.add)
                    else:
                        nc.gpsimd.tensor_tensor(out=p3, in0=e3, in1=g3,
                                                op=mybir.AluOpType.mult)
                        # add-tree over d on gpsimd; last step -> egs slice
                        wdt = D // 2
                        while wdt >= 1:
                            if wdt == 1:
                                out_ap = egs[rt][:, hb * 8:(hb + 1) * 8
                                                 ].rearrange(
                                    "p (k o) -> p k o", o=1)
                            else:
                                out_ap = p3[:, :, 0:wdt]
                            nc.gpsimd.tensor_tensor(
                                out=out_ap, in0=p3[:, :, 0:wdt],
                                in1=p3[:, :, wdt:2 * wdt],
                                op=mybir.AluOpType.add)
                            wdt //= 2

                    eoff = psmall.tile([P, 8], U32, tag="eoff")
                    nc.gpsimd.tensor_tensor(
                        out=eoff[:], in0=idx8,
                        in1=prow_t[:].to_broadcast([P, 8]),
                        op=mybir.AluOpType.add)
                    nc.gpsimd.indirect_dma_start(
                        out=a8s[rt][:, hb * 8:(hb + 1) * 8], out_offset=None,
                        in_=astage[(bu, rt)][:, :],
                        in_offset=bass.IndirectOffsetOnAxis(ap=eoff[:],
                                                            axis=1))
                    if h == H - 1 and bu == 1:
                        emit_tail(rt)

    nc.compile()
    return nc


def _pack_edges(src, dst):
    """Cluster edges by connected components, pack into 8 bins of 512."""
    E = len(src)
    parent = np.arange(N)

    def find(x):
        while parent[x] != x:
            parent[x] = parent[parent[x]]
            x = parent[x]
        return x

    for s, d in zip(src, dst):
        rs, rd = find(s), find(d)
        if rs != rd:
            parent[rs] = rd
    from collections import defaultdict
    comp = defaultdict(list)
    for e in range(E):
        comp[find(src[e])].append(e)
    bins = [[] for _ in range(NCORES)]
    cap = [NB] * NCORES
    for c in sorted(comp.values(), key=len, reverse=True):
        rem = c
        while rem:
            b = int(np.argmax(cap))
            take = min(cap[b], len(rem))
            bins[b].extend(rem[:take])
            cap[b] -= take
            rem = rem[take:]
    assert all(v == 0 for v in cap)
    return [np.array(b, dtype=np.int64) for b in bins]


def host_prep(embeds, field, uncertainty, adj, batch_edges):
    embeds = np.asarray(embeds, np.float32)
    field = np.asarray(field, np.float32)
    adj_u8 = (np.asarray(adj) != 0.0).astype(np.uint8)
    src = np.asarray(batch_edges[0]).astype(np.int64)
    dst = np.asarray(batch_edges[1]).astype(np.int64)

    bins = _pack_edges(src, dst)
    NT = 6
    for b in bins:
        nodes = np.unique(np.concatenate([src[b], dst[b]]))
        NT = max(NT, int(np.ceil(len(nodes) / P)))
    SLT = NT * P

    # doubled-K fp8 operand: slot0 = e rows, slot1 = -y2 split into 3 fp8
    # components on contraction rows 0..2 (error ~1e-2, better than bf16)
    y2 = (embeds.astype(np.float64) ** 2).sum(-1).astype(np.float32)  # [H,N]
    c1 = y2.astype(f8).astype(np.float32)
    c2 = (y2 - c1).astype(f8).astype(np.float32)
    c3 = (y2 - c1 - c2).astype(f8).astype(np.float32)
    eT8 = np.zeros((H, D, 2, N), dtype=f8)
    eT8[:, :, 0, :] = embeds.transpose(0, 2, 1).astype(f8)
    eT8[:, 0, 1, :] = (-c1).astype(f8)
    eT8[:, 1, 1, :] = (-c2).astype(f8)
    eT8[:, 2, 1, :] = (-c3).astype(f8)
    eT8 = eT8.reshape(H, D, 2 * N)
    emb_rows = [np.ascontiguousarray(embeds[hh]).astype(bf) for hh in range(H)]
    prow_np = (np.arange(P, dtype=np.uint32) * np.uint32(N)).reshape(P, 1)

    in_maps = []
    for m in range(NCORES):
        eb = bins[m]
        s_sh, d_sh = src[eb], dst[eb]
        nodes = {0: s_sh, 1: d_sh}
        nodelist = np.unique(np.concatenate([s_sh, d_sh]))
        n_c = len(nodelist)
        assert n_c <= SLT
        slots = np.zeros(SLT, dtype=np.int64)
        slots[:n_c] = nodelist
        slotmap = {int(nd): i for i, nd in enumerate(nodelist)}

        xT8_np = np.zeros((H, D, 2, SLT), dtype=f8)
        xT8_np[:, :, 0, :] = (2.0 * embeds[:, slots, :]).transpose(
            0, 2, 1).astype(f8)
        xT8_np[:, 0:3, 1, :] = 1.0
        xT8_np = xT8_np.reshape(H, D, 2 * SLT)

        grows_np = np.empty((H, 2, NB, D), dtype=bf)
        xg = np.empty((H, 2, NB), dtype=np.float32)
        tofs_np = np.empty((P, 2 * RT), dtype=np.uint32)
        eofs_np = np.empty((P, 2 * RT), dtype=np.uint32)
        for bu in range(2):
            grows_np[:, bu] = field[:, nodes[1 - bu], :].astype(bf)
            xg[:, bu] = np.einsum('hbd,hbd->hb', embeds[:, nodes[bu], :],
                                  field[:, nodes[1 - bu], :])
            eslot = np.array([slotmap[int(nd)] for nd in nodes[bu]],
                             dtype=np.uint32)
            p_s, nt_s = eslot % P, eslot // P
            row = p_s * NT + nt_s
            for rt in range(RT):
                rsl = slice(rt * P, (rt + 1) * P)
                tofs_np[:, bu * RT + rt] = row[rsl] * 8
                eofs_np[:, bu * RT + rt] = row[rsl] * KR
        xgp_np = np.ascontiguousarray(
            xg.reshape(8, NB).transpose(1, 0).reshape(RT, P, 8))

        im = {"eT8": eT8, "xT8": xT8_np, "xgp": xgp_np,
              "prow": prow_np, "tofs": tofs_np, "eofs": eofs_np,
              "grows": grows_np}
        for hh in range(H):
            im[f"emb_{hh}"] = emb_rows[hh]
        for rt in range(RT):
            rsl = slice(rt * P, (rt + 1) * P)
            im[f"astage_0_{rt}"] = np.ascontiguousarray(adj_u8[:, d_sh[rsl]].T)
            im[f"astage_1_{rt}"] = np.ascontiguousarray(adj_u8[s_sh[rsl], :])
        in_maps.append(im)
    return in_maps, bins, NT


_CACHE = {}


def _ensure_ntff_hook():
    import types

    try:
        import antenv.axon_hooks  # noqa: F401
    except ImportError:
        mod = types.ModuleType('antenv.axon_hooks')
        mod._hook = None
        mod.set_axon_ntff_profile_hook = lambda h: setattr(mod, '_hook', h)
        mod.get_axon_ntff_profile_hook = lambda: mod._hook
        import antenv
        antenv.axon_hooks = mod
        sys.modules['antenv.axon_hooks'] = mod
    from antenv.axon_hooks import (get_axon_ntff_profile_hook,
                                   set_axon_ntff_profile_hook)
    if get_axon_ntff_profile_hook() is None:
        from trn_agent_boot.trn_boot import _ntff_profile_via_ctypes
        set_axon_ntff_profile_hook(
            _ntff_profile_via_ctypes('/opt/axon/libaxon_pjrt.so'))


def kernel(embeds, field, uncertainty, adj, batch_edges, _profile=None):
    """Full inputs in, full (4096,) f32 output. Runs on NeuronCores 0-7."""
    if _profile:
        try:
            _ensure_ntff_hook()
        except Exception as ex:
            print(f"(ntff hook registration failed: {ex})")
    u = float(np.asarray(uncertainty).reshape(-1)[0])
    in_maps, bins, NT = host_prep(embeds, field, uncertainty, adj,
                                  batch_edges)
    if ('nc', u, NT) not in _CACHE:
        _CACHE[('nc', u, NT)] = build_kernel(u, NT)
    nc = _CACHE[('nc', u, NT)]
    res = run_bass_kernel_spmd(nc, in_maps, list(range(NCORES)),
                               trace=bool(_profile))
    if isinstance(_profile, dict):
        _profile['exec_time_ns'] = res.exec_time_ns
        _profile['res'] = res
    out = np.empty(B, dtype=np.float32)
    for m in range(NCORES):
        out[bins[m]] = np.asarray(res.results[m]["out"],
                                  np.float32).reshape(-1)
    return out
